# revision 19
# baseline (speedup 1.0000x reference)
"""Trainium2 Bass kernel for block-local MultiHeadAttention + output projection.

Reference computation (per batch b):
  Q = x @ Wq.T ; K = x @ Wk.T ; V = x @ Wv.T          x: [B, S=8192, 64]
  reshape to [B, G=512, H=16, 64] (groups of 16 consecutive tokens)
  E[g,h,k] = Q[g,h,:] . K[g,k,:]                      (16x16 block-diag attention)
  A = softmax(E / 32, axis=k)
  O[g,h,:] = sum_k A[g,h,k] V[g,k,:]
  out2[b, r, gm*64+d] = O[g=(gq,gm), h, d]  with r = h*32+gq
  y = out2 @ Wo.T + bo                                y: [B, 512, 1024]

Kernel strategy (data-parallel over batch, 4 batches/core on 8 cores):
  - M^T = Wk^T Wq so that E[h,k] = X_h . Z_k with Z = X M^T  (skips Q,K)
  - x DMA'd in slab-token order: XB[p = k*8+j, (b,q | gm | d)] with 256B
    contiguous runs (token t = ((q*8+j)*16+gm)*16+k); XB doubles as the
    O-matmul stationary (no separate XPP pass)
  - XT slab pairs via paired-q PE transposes of XB 3D slices [128,(2q),64]:
    XT[(q%2)*64+d, ((b*16+gm)*2+q//2)*128 + k*8+j] - contiguous evictions
  - ZT = M X^T via full-array blockdiag(M) matmuls (both parities at once)
  - per "slab" (b, gm, q) = 8 groups {gq = q*8+j} x 16 tokens:
    E^T-matmuls for a gm PAIR packed in one [128,1024] PSUM tile, one exp
    per pair, blockdiag mask kron(ones16, eye8), den via ones-matmul,
    U^T assembled directly as out2^T chunks in PSUM
  - normalization: approx-reciprocal (DVE custom op) + mul on eviction
  - Wv folded into Wo: WoV[:, gm-block] = Wo[:, gm-block] @ Wv
  - fc: y-tile = (out2^T-tile stationary) @ WoV^T streaming; bias added on
    the PSUM eviction (DVE add with broadcast-materialized bias)
  - weight-setup (M, WoV, masks) is emitted once, outside the timing loop
"""

import numpy as np
from contextlib import ExitStack

import concourse.bass as bass
import concourse.bacc as bacc
import concourse.mybir as mybir
import concourse.tile as tile

N_CORES = 8
B_GLOB = 32
B_LOC = B_GLOB // N_CORES   # 4 batches per core
SB = 8192                   # tokens per batch
D = 64                      # head dim
NG = 16                     # gm values (heads)
NQ = 4                      # gq octs per batch
NJ = 8                      # groups per slab
NH = 16                     # tokens per group
E = 1024
RB = 512                    # out2 rows per batch
TOK = B_LOC * SB            # 32768 tokens per core

BF = mybir.dt.bfloat16
F32 = mybir.dt.float32
AF = mybir.ActivationFunctionType


def slab_xt_ap(T, b, gm, q):
    """[64@(q%2), 128] contiguous view of slab (b,gm,q) in XT/ZT layout:
    col = ((b*16+gm)*2 + q//2)*128 + k*8 + j, rows (q%2)*64 + d."""
    half = (q % 2) * 64
    pair = (b * 16 + gm) * 2 + q // 2
    return T[half:half + 64, pair * 128:(pair + 1) * 128]


def make_persist(ctx, tc):
    pp = ctx.enter_context(tc.tile_pool(name="persist", bufs=1))
    P = {}
    P["XPP"] = pp.tile([128, B_LOC * NG * NQ * D], BF, tag="XPP", name="XPP")
    P["XT"] = pp.tile([128, 8 * NH * 128], BF, tag="XT", name="XT")
    P["ZT"] = pp.tile([128, 8 * NH * 128], BF, tag="ZT", name="ZT")
    P["WOVT"] = pp.tile([128, 8 * E], BF, tag="WOVT", name="WOVT")
    P["OUT2T"] = pp.tile([128, B_LOC * 8 * RB], BF, tag="OUT2T", name="OUT2T")
    P["MASK2"] = pp.tile([128, 1024], BF, tag="MASK2", name="MASK2")
    P["ONES64"] = pp.tile([128, D], BF, tag="ONES64", name="ONES64")
    P["ONESROW"] = pp.tile([1, 128], BF, tag="ONESROW", name="ONESROW")
    P["IDN"] = pp.tile([128, 128], BF, tag="IDN", name="IDN")
    P["M2"] = pp.tile([128, 128], BF, tag="M2", name="M2")
    P["WV2"] = pp.tile([128, D], BF, tag="WV2", name="WV2")
    P["BOFULL"] = pp.tile([128, E], BF, tag="BOFULL", name="BOFULL")
    return P


def emit_setup(tc, P, ins):
    """One-time weight transforms (outside the timing loop)."""
    nc = tc.nc
    x, wq, wk, wv, wo, bo = ins
    with tc.tile_pool(name="setup", bufs=1) as sp, \
         tc.tile_pool(name="setup_ps", bufs=2, space="PSUM") as spp:
        nc.vector.memset(P["ONES64"][:], 1.0)
        nc.vector.memset(P["ONESROW"][:], 1.0)
        nc.vector.memset(P["MASK2"][:], 0.0)
        ones128 = sp.tile([128, 128], BF, tag="ones128")
        nc.vector.memset(ones128[:], 1.0)
        nc.gpsimd.affine_select(P["IDN"][:], ones128[:], pattern=[[1, 128]],
                                compare_op=mybir.AluOpType.is_equal, fill=0.0,
                                base=0, channel_multiplier=-1)
        # mask rows (k*8+j), cols blk*128 + h*8 + j2 ; valid iff j == j2
        rp = sp.tile([8, 1024], BF, tag="rp")
        nc.vector.memset(rp[0:8, :], 0.0)
        for j in range(NJ):
            dst = rp[j:j + 1, :].rearrange("p (qk j2) -> j2 p qk", j2=8)[j]
            nc.sync.dma_start(dst, ones128[0:1, :])
        for k in range(16):
            nc.sync.dma_start(P["MASK2"][k * 8:(k + 1) * 8, :], rp[0:8, :])
        # small weights
        wq_st = sp.tile([64, 64], F32, tag="wq_st")
        wk_st = sp.tile([64, 64], F32, tag="wk_st")
        wv_st = sp.tile([128, 64], F32, tag="wv_st")
        nc.sync.dma_start(wq_st[:], wq)
        nc.sync.dma_start(wk_st[:], wk)
        nc.sync.dma_start(wv_st[0:64, :], wv)
        nc.sync.dma_start(wv_st[64:128, :], wv)
        nc.vector.tensor_copy(P["WV2"][:], wv_st[:])
        # M^T = Wk^T @ Wq, block-diagonal duplicated: M2[0:64,0:64]=M2[64:,64:]=M^T
        mt_ps = spp.tile([128, 128], F32, tag="mt_ps")
        nc.tensor.matmul(mt_ps[0:64, 0:64], wk_st[:], wq_st[:],
                         start=True, stop=True)
        nc.tensor.matmul(mt_ps[64:128, 64:128], wk_st[:], wq_st[:],
                         start=True, stop=True, tile_position=(0, 64))
        nc.vector.memset(P["M2"][:], 0.0)
        nc.vector.tensor_copy(P["M2"][0:64, 0:64], mt_ps[0:64, 0:64])
        nc.vector.tensor_copy(P["M2"][64:128, 64:128], mt_ps[64:128, 64:128])

        # bias broadcast: BOFULL[p, e] = bo[e]
        bo_st = sp.tile([1, E], F32, tag="bo_st")
        nc.sync.dma_start(bo_st[:], bo.rearrange("(p n) -> p n", p=1))
        bo_bf = sp.tile([1, E], BF, tag="bo_bf")
        nc.vector.tensor_copy(bo_bf[:], bo_st[:])
        for half in range(2):
            bops = spp.tile([128, 512], F32, tag="bops")
            nc.tensor.matmul(bops[:], P["ONESROW"][0:1, :],
                             bo_bf[:, half * 512:(half + 1) * 512],
                             start=True, stop=True)
            nc.vector.tensor_copy(P["BOFULL"][:, half * 512:(half + 1) * 512],
                                  bops[:])

        # Wo load + cast + transpose (PE) + fold Wv -> WOVT
        wot = sp.tile([128, 8 * E], BF, tag="wot")   # Wo^T chunks [e', e]
        for t in range(8):
            wo_st = sp.tile([128, E], F32, tag="wo_st")
            nc.sync.dma_start(wo_st[:], wo[t * 128:(t + 1) * 128, :])
            wo_bf = sp.tile([128, E], BF, tag="wo_bf")
            nc.vector.tensor_copy(wo_bf[:], wo_st[:])
            for c in range(8):
                tp = spp.tile([128, 512], BF, tag="wo_tp")
                nc.tensor.transpose(tp[:, 0:128],
                                    wo_bf[:, c * 128:(c + 1) * 128], P["IDN"][:])
                nc.vector.tensor_copy(
                    wot[:, c * E + t * 128: c * E + (t + 1) * 128], tp[:, 0:128])
        for c2 in range(8):
            for half in range(2):
                wov_ps = spp.tile([128, 512], F32, tag="wov_ps")
                for gmh in range(2):
                    gm = c2 * 2 + gmh
                    pb = (gm % 2) * 64
                    nc.tensor.matmul(
                        wov_ps[pb:pb + 64, :],
                        P["WV2"][pb:pb + 64, :],
                        wot[pb:pb + 64, (gm // 2) * E + half * 512:
                            (gm // 2) * E + half * 512 + 512],
                        start=True, stop=True,
                        tile_position=(pb, pb),
                    )
                nc.vector.tensor_copy(
                    P["WOVT"][:, c2 * E + half * 512: c2 * E + half * 512 + 512],
                    wov_ps[:])


def emit_main(ctx, tc, P, ins, outs, dbg, stage=99):
    nc = tc.nc
    x = ins[0]
    y = outs["y"]
    XPP, XT, ZT, WOVT, OUT2T = P["XPP"], P["XT"], P["ZT"], P["WOVT"], P["OUT2T"]

    if stage < 2:
        return
    xl = ctx.enter_context(tc.tile_pool(name="xload", bufs=3))
    wps = ctx.enter_context(tc.tile_pool(name="wps", bufs=2, space="PSUM"))
    eps_pool = ctx.enter_context(tc.tile_pool(name="eps", bufs=2, space="PSUM"))
    dps_pool = ctx.enter_context(tc.tile_pool(name="dps", bufs=1, space="PSUM"))
    ops_pool = ctx.enter_context(tc.tile_pool(name="ops", bufs=1, space="PSUM"))
    aex_pool = ctx.enter_context(tc.tile_pool(name="aex", bufs=3))
    am_pool = ctx.enter_context(tc.tile_pool(name="am", bufs=3))
    rden_pool = ctx.enter_context(tc.tile_pool(name="rden", bufs=3))
    fout_pool = ctx.enter_context(tc.tile_pool(name="fout", bufs=2))

    xb_pool = ctx.enter_context(tc.tile_pool(name="xb16", bufs=2))

    for b in range(B_LOC):
        # ---- x load (4KB runs: p = j*16+gm, free (q|k|d)) + cast ----
        srcs = x[b].rearrange("(n p m) d -> n p (m d)", p=128, m=16)
        XB16 = xb_pool.tile([128, NQ * NH * D], BF, tag="XB16")
        for q in range(NQ):
            st = xl.tile([128, NH * D], F32, tag="xstage")
            nc.sync.dma_start(st[:], srcs[q])
            nc.gpsimd.tensor_copy(XB16[:, q * NH * D:(q + 1) * NH * D], st[:])
        if stage < 3:
            continue

        # ---- XT via PE transposes of XB16 [128,64] slices + scatter ----
        for Q in range(2):          # q pair
            for hb in range(4):     # 4 banks of 4 h each
                tp = wps.tile([128, 512], BF, tag="wps", name="tp")
                for hh in range(4):
                    h = hb * 4 + hh
                    for qp in range(2):
                        q = Q * 2 + qp
                        src = XB16[:, (q * NH + h) * D:(q * NH + h + 1) * D]
                        nc.tensor.transpose(
                            tp[qp * 64:(qp + 1) * 64, hh * 128:(hh + 1) * 128],
                            src, P["IDN"][:],
                            tile_position=(0, qp * 64))
                # scatter (hh | j, gm) -> XT col (b,gm)*256 + Q*128 + h*8+j
                dst = XT[:].rearrange(
                    "p (bb gm q2 hb2 hh j) -> bb q2 hb2 p hh j gm",
                    bb=B_LOC, gm=NG, q2=2, hb2=4, hh=4, j=NJ)[b, Q, hb]
                nc.vector.tensor_copy(dst, tp[:])
        if stage < 4:
            continue

        # ---- ZT = M X^T (full-array blockdiag matmuls) ----
        for r in range(8):
            zps = wps.tile([128, 512], F32, tag="wps", name="zps")
            coff = b * 4096 + r * 512
            nc.tensor.matmul(zps[:], P["M2"][:], XT[:, coff:coff + 512],
                             start=True, stop=True)
            nc.scalar.copy(ZT[:, coff:coff + 512], zps[:])
        if stage < 4.5:
            continue

        # ---- XPP via PE transposes of XT slabs ----
        for gq4 in range(4):        # gm quad
            tpa = wps.tile([128, 512], BF, tag="wps", name="tpa")
            tpb = wps.tile([128, 512], BF, tag="wps", name="tpb")
            for gml in range(4):
                gm = gq4 * 4 + gml
                for q in range(NQ):
                    srcx = slab_xt_ap(XT, b, gm, q)
                    half = (q % 2) * 64
                    dstp = tpa if q % 2 == 0 else tpb
                    col = (gml * 2 + q // 2) * 64
                    nc.tensor.transpose(
                        dstp[:, col:col + 64], srcx,
                        P["IDN"][half:half + 64, half:half + 64],
                        tile_position=(half, 0))
            base = ((b * 16 + gq4 * 4) * 4) * D
            dsta = XPP[:, base: base + 16 * D].rearrange(
                "p (gml q2 par d) -> par p gml q2 d",
                gml=4, q2=2, par=2, d=D)
            nc.vector.tensor_copy(dsta[0], tpa[:])
            nc.vector.tensor_copy(dsta[1], tpb[:])
        if stage < 5:
            continue

        # ---- attention, one gm pair at a time ----
        for c in range(8):
            dps = dps_pool.tile([128, 512], F32, tag="dps")
            ops = ops_pool.tile([128, 512], F32, tag="ops")
            eps = eps_pool.tile([128, 1024], F32, tag="eps")
            for gg in range(2):
                gm = c * 2 + gg
                for q in range(NQ):
                    half = (q % 2) * 64
                    col = (q % 2) * 512 + gg * 256 + (q // 2) * 128
                    nc.tensor.matmul(
                        eps[:, col:col + 128],
                        slab_xt_ap(ZT, b, gm, q),
                        slab_xt_ap(XT, b, gm, q),
                        start=True, stop=True,
                        tile_position=(half, 0),
                    )
            if stage < 5.2:
                continue
            aex = aex_pool.tile([128, 1024], BF, tag="aex")
            nc.scalar.activation(aex[:], eps[:], AF.Exp, scale=1.0 / 32.0)
            if stage < 5.4:
                continue
            am = am_pool.tile([128, 1024], BF, tag="am")
            nc.vector.tensor_mul(am[:], aex[:], P["MASK2"][:])
            if stage < 5.6:
                continue
            for gg in range(2):
                gm = c * 2 + gg
                pb = gg * 64
                for q in range(NQ):
                    aoff = (q % 2) * 512 + gg * 256 + (q // 2) * 128
                    rhs = am[:, aoff:aoff + 128]
                    nc.tensor.matmul(dps[pb:pb + 64, q * 128:(q + 1) * 128],
                                     P["ONES64"][:], rhs, start=True, stop=True,
                                     tile_position=(0, pb))
                    sidx = (b * 16 + gm) * 4 + q
                    nc.tensor.matmul(ops[pb:pb + 64, q * 128:(q + 1) * 128],
                                     XPP[:, sidx * D:(sidx + 1) * D], rhs,
                                     start=True, stop=True, tile_position=(0, pb))
            if stage < 5.8:
                continue
            rden = rden_pool.tile([128, 512], F32, tag="rden")
            nc.vector.reciprocal_approx_fast(rden[:], dps[:])
            sec = (b * 8 + c) * 512
            out_ap = OUT2T[:, sec:sec + 512].rearrange(
                "p (h q2 j) -> p q2 h j", h=NH, q2=NQ, j=NJ)
            nc.vector.tensor_mul(out_ap, ops[:], rden[:])

        # ---- fc for this batch ----
        if stage < 6:
            continue
        for rt in range(4):
            fo = fout_pool.tile([128, E], F32, tag="fout")
            for halfe in range(2):
                fps = wps.tile([128, 512], F32, tag="wps", name="fps")
                for c in range(8):
                    sec = (b * 8 + c) * 512
                    nc.tensor.matmul(
                        fps[:],
                        OUT2T[:, sec + rt * 128: sec + (rt + 1) * 128],
                        WOVT[:, c * E + halfe * 512: c * E + halfe * 512 + 512],
                        start=(c == 0), stop=(c == 7),
                    )
                nc.vector.tensor_add(fo[:, halfe * 512:(halfe + 1) * 512],
                                     fps[:],
                                     P["BOFULL"][:, halfe * 512:(halfe + 1) * 512])
            row = b * RB + rt * 128
            nc.sync.dma_start(y[row:row + 128, :], fo[:])

    # ---------------- debug dumps ----------------
    for name, T in (("xpp", XPP), ("xt", XT), ("zt", ZT), ("out2t", OUT2T)):
        if name in dbg:
            nc.sync.dma_start(dbg[name], T[:])


def build(reps=1, debug=(), stage=99):
    nc = bacc.Bacc("TRN2", target_bir_lowering=False, debug=False,
                   num_devices=N_CORES)
    x = nc.dram_tensor("x", [B_LOC, SB, D], F32, kind="ExternalInput").ap()
    wq = nc.dram_tensor("wq", [D, D], F32, kind="ExternalInput").ap()
    wk = nc.dram_tensor("wk", [D, D], F32, kind="ExternalInput").ap()
    wv = nc.dram_tensor("wv", [D, D], F32, kind="ExternalInput").ap()
    wo = nc.dram_tensor("wo", [E, E], F32, kind="ExternalInput").ap()
    bo = nc.dram_tensor("bo", [E], F32, kind="ExternalInput").ap()
    y = nc.dram_tensor("y", [B_LOC * RB, E], F32, kind="ExternalOutput").ap()
    dbg = {}
    for name, shape, dt in [
        ("xpp", [128, B_LOC * NG * NQ * D], BF),
        ("xt", [128, 8 * NH * 128], BF),
        ("zt", [128, 8 * NH * 128], BF),
        ("out2t", [128, B_LOC * 8 * RB], BF),
    ]:
        if name in debug:
            dbg[name] = nc.dram_tensor(name, shape, dt, kind="ExternalOutput").ap()

    ins = (x, wq, wk, wv, wo, bo)
    outs = {"y": y}
    with tile.TileContext(nc) as tc:
        with ExitStack() as ctx:
            P = make_persist(ctx, tc)
            if stage >= 1:
                emit_setup(tc, P, ins)
            if reps > 1:
                with tc.For_i(0, reps, 1):
                    emit_main(ctx, tc, P, ins, outs, dbg, stage=stage)
            else:
                emit_main(ctx, tc, P, ins, outs, dbg, stage=stage)
    nc.compile()
    return nc


def kernel(x, Wq, Wk, Wv, Wo, bo):
    """Full-input entry point: shards batch over 8 cores, returns full output."""
    from concourse.bass_utils import run_bass_kernel_spmd

    nc = build()
    in_maps = []
    for core in range(N_CORES):
        xs = np.ascontiguousarray(x[core * B_LOC:(core + 1) * B_LOC])
        in_maps.append({
            "x": xs, "wq": np.asarray(Wq), "wk": np.asarray(Wk),
            "wv": np.asarray(Wv), "wo": np.asarray(Wo), "bo": np.asarray(bo),
        })
    res = run_bass_kernel_spmd(nc, in_maps, list(range(N_CORES)))
    out = np.concatenate([res.results[c]["y"] for c in range(N_CORES)], axis=0)
    return out.reshape(B_GLOB, RB, E)
